# revision 19
# baseline (speedup 1.0000x reference)
"""Causal self-attention (GQA + RoPE + QK-RMSNorm) on 8 trn2 NeuronCores.

Reference (B=2, T=2048, C=2048, 16 q-heads / 4 kv-heads, head_dim 128):
    q = rms_norm(rope(x @ Wq)) / sqrt(128); k = rms_norm(rope(x @ Wk))
    att = softmax_causal(q k^T / sqrt(128)); y = (att @ v) @ Wp

Sharding: core = 4*b + g  (b = batch 0..1, g = head-group 0..3).
Each core computes q-heads 4g..4g+3 (all mapping to kv-head g under GQA),
attends over the full causal sequence of its batch, the 4 cores of a batch
AllGather their attention outputs per head, and each computes a distinct
512-column slice of the output projection. Host concatenates.

v2 performance notes vs the v1 baseline:
- Q/K projections run in fp8 (e4m3) with DoubleRow perf mode (256-deep
  contraction per matmul, ~1.4x PE rate). Weights are pre-scaled x16 on the
  host so fp8 sees a healthy range; the scale cancels exactly inside the
  q/k RMS norms (rotation preserves norms, and the norm epsilon terms are
  adjusted by 256 to keep the math identical).
- K's rms-norm reciprocal is folded into K-hat on chip, so the attention
  exp() needs no per-partition scale and can process two key blocks per
  activation instruction ([128,1024]), amortizing ACT overhead.
- RoPE and elementwise work run on bf16 SBUF tiles (2x DVE rate); psum
  evacuation copies and squares run on the scalar engine.
- The output projection is split in two passes interleaved between
  attention heads so the AllGathers overlap compute.
"""

import ml_dtypes
import numpy as np

B, T, C = 2, 2048, 2048
NH, NKV, HD = 16, 4, 128
G = 4  # q-heads per core
EPS = 1e-6
WS = 16.0  # host-side fp8 weight prescale (cancels in rms-norm)
NKB = C // 256  # 8 double-row contraction blocks
NCB = C // 128  # 16 single contraction blocks
NTKB = T // 128  # 16 key blocks
F8 = ml_dtypes.float8_e4m3
BF = ml_dtypes.bfloat16

_CACHE = {}


def _build():
    import concourse.mybir as mybir
    import concourse.tile as tile
    from concourse import bacc
    from concourse.masks import make_identity
    from contextlib import ExitStack

    F32 = mybir.dt.float32
    BF16 = mybir.dt.bfloat16
    FP8 = mybir.dt.float8e4
    AF = mybir.ActivationFunctionType
    DR = mybir.MatmulPerfMode.DoubleRow

    nc = bacc.Bacc(None, target_bir_lowering=False, num_devices=8)

    x8 = nc.dram_tensor("x8", [NKB, 128, 2, T], FP8, kind="ExternalInput")
    xT = nc.dram_tensor("xT", [C, T], BF16, kind="ExternalInput")
    wq8 = nc.dram_tensor("wq8", [NKB, 128, 2, G * HD], FP8, kind="ExternalInput")
    wk8 = nc.dram_tensor("wk8", [NKB, 128, 2, HD], FP8, kind="ExternalInput")
    wv = nc.dram_tensor("wv", [C, HD], BF16, kind="ExternalInput")
    wp = nc.dram_tensor("wp", [C, G * HD], BF16, kind="ExternalInput")
    cosb = nc.dram_tensor("cosb", [128, T], BF16, kind="ExternalInput")
    sinb = nc.dram_tensor("sinb", [128, T], BF16, kind="ExternalInput")
    masks = nc.dram_tensor("masks", [4, 128, 512], BF16, kind="ExternalInput")
    outT = nc.dram_tensor("outT", [G * HD, T], F32, kind="ExternalOutput")

    with tile.TileContext(nc) as tc:
        with ExitStack() as outer:
            dram = outer.enter_context(tc.tile_pool(name="dram", bufs=1, space="DRAM"))
            ag_in = dram.tile([G * HD, T], BF16)
            ag_outs = [
                dram.tile([4 * HD, T], BF16, name=f"ag_out_{q}") for q in range(3)
            ]
            ag_in3 = [
                dram.tile([HD, T // 2], BF16, name=f"ag_in3_{h}") for h in range(2)
            ]
            ag_in0 = [
                dram.tile([HD, T // 2], BF16, name=f"ag_in0_{h}") for h in range(2)
            ]
            ag_out0 = [
                dram.tile([4 * HD, T // 2], BF16, name=f"ag_out0_{h}")
                for h in range(2)
            ]
            ag_out3 = [
                dram.tile([4 * HD, T // 2], BF16, name=f"ag_out3_{h}")
                for h in range(2)
            ]

            consts = outer.enter_context(tc.tile_pool(name="consts", bufs=1))
            ones_bf = consts.tile([128, 1], BF16)
            nc.vector.memset(ones_bf[:], 1.0)
            ones_row = consts.tile([1, 128], BF16)
            nc.vector.memset(ones_row[:], 1.0)
            ident_bf = consts.tile([128, 128], BF16)
            make_identity(nc, ident_bf[:])
            eps_k1 = consts.tile([1, 1], F32)
            nc.vector.memset(eps_k1[:], 256.0 * EPS)
            eps_q1 = consts.tile([1, 1], F32)
            nc.vector.memset(eps_q1[:], 256.0 * HD * HD * EPS)

            wpool = outer.enter_context(tc.tile_pool(name="w", bufs=1))
            wq_sb = wpool.tile([128, NKB, 2, G * HD], FP8)
            wk_sb = wpool.tile([128, NKB, 2, HD], FP8)
            wv_sb = wpool.tile([128, NCB, HD], BF16)
            wp_sb = wpool.tile([128, NCB, G * HD], BF16)
            for kb in range(NKB):
                nc.sync.dma_start(out=wk_sb[:, kb], in_=wk8[kb])
            for cb in range(NCB):
                nc.gpsimd.dma_start(
                    out=wv_sb[:, cb, :], in_=wv[128 * cb : 128 * cb + 128, :]
                )
            for kb in range(NKB):
                nc.sync.dma_start(out=wq_sb[:, kb], in_=wq8[kb])
            for cb in range(NCB):
                nc.gpsimd.dma_start(
                    out=wp_sb[:, cb, :], in_=wp[128 * cb : 128 * cb + 128, :]
                )

            trig = outer.enter_context(tc.tile_pool(name="trig", bufs=1))
            cos_sb = trig.tile([128, T], BF16)
            sin_sb = trig.tile([128, T], BF16)
            masks_sb = trig.tile([128, 4, 512], BF16)
            nc.gpsimd.dma_start(out=cos_sb[:], in_=cosb[:])
            nc.gpsimd.dma_start(out=sin_sb[:], in_=sinb[:])
            nc.gpsimd.dma_start(
                out=masks_sb[:], in_=masks.rearrange("d p m -> p d m")
            )

            acts = outer.enter_context(tc.tile_pool(name="acts", bufs=1))
            qhT = acts.tile([128, G, T], BF16)  # normalized rope'd q, [d, t]
            khT = acts.tile([128, T], BF16)  # normalized rope'd k, [d, t]
            v_sb = acts.tile([128, NTKB, HD], BF16)  # v transposed to [t, d]

            # ---- phase 1: projections + RoPE + RMS-norm, 1024-col chunks ----
            with ExitStack() as s1:
                x8_pool = s1.enter_context(tc.tile_pool(name="x8", bufs=16))
                xt_pool = s1.enter_context(tc.tile_pool(name="xt", bufs=24))
                tmp = s1.enter_context(tc.tile_pool(name="tmp", bufs=2))
                bcp = s1.enter_context(tc.tile_pool(name="bcp", bufs=3))
                rrp = s1.enter_context(tc.tile_pool(name="rrp", bufs=3))
                psbig = s1.enter_context(
                    tc.tile_pool(name="psbig", bufs=2, space="PSUM")
                )
                psrow = s1.enter_context(
                    tc.tile_pool(name="psrow", bufs=2, space="PSUM")
                )
                pstr = s1.enter_context(tc.tile_pool(name="pstr", bufs=1, space="PSUM"))

                def rope_a(ps_src, tcs, name):
                    """rope(ps_src) on bf16 tiles + column-norm squares.
                    Returns (rotd, sq)."""
                    src_bf = tmp.tile([128, 1024], BF16, tag="r_bf", name=f"{name}_bf")
                    nc.scalar.copy(out=src_bf[:], in_=ps_src[:])
                    rot = tmp.tile([128, 1024], BF16, tag="r_rot", name=f"{name}_rot")
                    nc.vector.tensor_copy(out=rot[0:64, :], in_=src_bf[64:128, :])
                    nc.vector.tensor_copy(out=rot[64:128, :], in_=src_bf[0:64, :])
                    qr = tmp.tile([128, 1024], BF16, tag="r_qr", name=f"{name}_qr")
                    nc.vector.tensor_mul(qr[:], src_bf[:], cos_sb[:, tcs])
                    nc.vector.tensor_mul(rot[:], rot[:], sin_sb[:, tcs])
                    rotd = tmp.tile([128, 1024], BF16, tag="r_rd", name=f"{name}_rd")
                    nc.vector.tensor_add(rotd[:], qr[:], rot[:])
                    sq = tmp.tile([128, 1024], BF16, tag="r_sq", name=f"{name}_sq")
                    nc.vector.tensor_mul(sq[:], rotd[:], rotd[:])
                    return rotd, sq

                def rope_b(dst, ab, sq_scale, sq_bias):
                    """dst = rotd * rsqrt-norm-broadcast."""
                    rotd, sq = ab
                    bc = bcp.tile([128, 1024], BF16, tag="bc")
                    for th in range(2):
                        hs = slice(512 * th, 512 * th + 512)
                        ps_row = psrow.tile([1, 512], F32, tag="psrow")
                        nc.tensor.matmul(
                            ps_row[:], ones_bf[:], sq[:, hs], start=True, stop=True
                        )
                        srow = rrp.tile([1, 512], F32, tag="srow")
                        nc.scalar.activation(
                            out=srow[:], in_=ps_row[:], func=AF.Sqrt,
                            scale=sq_scale, bias=sq_bias,
                        )
                        rr32 = rrp.tile([1, 512], F32, tag="rr32")
                        nc.vector.reciprocal_approx_fast(out=rr32[:], in_=srow[:])
                        rrbf = rrp.tile([1, 512], BF16, tag="rrbf")
                        nc.vector.tensor_copy(out=rrbf[:], in_=rr32[:])
                        nc.gpsimd.partition_broadcast(bc[:, hs], rrbf[:])
                    nc.vector.tensor_mul(dst, rotd[:], bc[:])

                for tch in range(2):
                    tcs = slice(1024 * tch, 1024 * tch + 1024)
                    x8s = []
                    for kb in range(NKB):
                        t8 = x8_pool.tile(
                            [128, 2, 1024], FP8, tag="x8", name=f"x8_{tch}_{kb}"
                        )
                        nc.scalar.dma_start(out=t8[:], in_=x8[kb, :, :, tcs])
                        x8s.append(t8)
                    xts = []
                    for cb in range(NCB):
                        xt_t = xt_pool.tile(
                            [128, 1024], BF16, tag="xt", name=f"xt{tch}_{cb}"
                        )
                        nc.sync.dma_start(
                            out=xt_t[:], in_=xT[128 * cb : 128 * cb + 128, tcs]
                        )
                        xts.append(xt_t)

                    # K projection (fp8 DoubleRow, two 512 halves)
                    ps_k = psbig.tile([128, 1024], F32, tag="ps", name="ps_k")
                    for th in range(2):
                        hs = slice(512 * th, 512 * th + 512)
                        for kb in range(NKB):
                            nc.tensor.matmul(
                                ps_k[:, hs], wk_sb[:, kb], x8s[kb][:, :, hs],
                                start=(kb == 0), stop=(kb == NKB - 1),
                                perf_mode=DR,
                            )
                    ka = rope_a(ps_k, tcs, "k")

                    # Q projections per head (fp8 DoubleRow), B-parts pipelined
                    prev = None

                    def flush_prev(p):
                        which, ab = p
                        if which == "k":
                            rope_b(khT[:, tcs], ab, 1.0 / HD, eps_k1[:])
                        else:
                            rope_b(
                                qhT[:, which, tcs], ab, float(HD), eps_q1[:]
                            )

                    prev = ("k", ka)
                    for hq in range(G):
                        ps_q = psbig.tile([128, 1024], F32, tag="ps", name="ps_q")
                        for th in range(2):
                            hs = slice(512 * th, 512 * th + 512)
                            for kb in range(NKB):
                                nc.tensor.matmul(
                                    ps_q[:, hs],
                                    wq_sb[:, kb, :, 128 * hq : 128 * hq + 128],
                                    x8s[kb][:, :, hs],
                                    start=(kb == 0), stop=(kb == NKB - 1),
                                    perf_mode=DR,
                                )
                        flush_prev(prev)
                        prev = (hq, rope_a(ps_q, tcs, f"q{hq}"))

                    # V projection (bf16) + transpose to [t, d]
                    ps_v = psbig.tile([128, 1024], F32, tag="ps", name="ps_v")
                    for th in range(2):
                        hs = slice(512 * th, 512 * th + 512)
                        for cb in range(NCB):
                            nc.tensor.matmul(
                                ps_v[:, hs], wv_sb[:, cb, :], xts[cb][:, hs],
                                start=(cb == 0), stop=(cb == NCB - 1),
                            )
                    flush_prev(prev)
                    v_bf = tmp.tile([128, 1024], BF16, tag="v_bf")
                    nc.scalar.copy(out=v_bf[:], in_=ps_v[:])
                    for tt in range(8):
                        ps_tr = pstr.tile([128, 128], BF16, tag="pstr")
                        nc.tensor.transpose(
                            ps_tr[:], v_bf[:, 128 * tt : 128 * tt + 128], ident_bf[:]
                        )
                        nc.vector.tensor_copy(
                            out=v_sb[:, 8 * tch + tt, :], in_=ps_tr[:]
                        )

            # ---- phase 2: attention + interleaved output projection ----
            with ExitStack() as s2:
                pt_pool = s2.enter_context(tc.tile_pool(name="pt", bufs=5))
                bc2 = s2.enter_context(tc.tile_pool(name="bc2", bufs=4))
                rr2 = s2.enter_context(tc.tile_pool(name="rr2", bufs=4))
                yt_pool = s2.enter_context(tc.tile_pool(name="yt", bufs=24))
                acc_pool = s2.enter_context(tc.tile_pool(name="acc", bufs=16))
                o_pool = s2.enter_context(tc.tile_pool(name="o", bufs=3))
                pss = s2.enter_context(tc.tile_pool(name="pss", bufs=2, space="PSUM"))
                psy = s2.enter_context(tc.tile_pool(name="psy", bufs=2, space="PSUM"))
                psrs = s2.enter_context(
                    tc.tile_pool(name="psrs", bufs=2, space="PSUM")
                )

                def attn_head(hq):
                    for tqc in range(4):
                        tqs = slice(512 * tqc, 512 * tqc + 512)
                        nblk = 4 * tqc + 4
                        ngr = nblk // 2
                        ps_y = psy.tile([128, 512], F32, tag="psy")
                        ps_rs = psrs.tile([1, 512], F32, tag="psrs")
                        pending = []

                        def flush():
                            pT, b0 = pending.pop(0)
                            pair = pt_pool.tile(
                                [128, 512], BF16, tag="pair", bufs=4
                            )
                            nc.vector.tensor_add(
                                pair[:], pT[:, 0:512], pT[:, 512:1024]
                            )
                            nc.tensor.matmul(
                                ps_rs[:], ones_bf[:], pair[:],
                                start=(b0 == 0), stop=(b0 + 2 == nblk),
                            )
                            for i in range(2):
                                b = b0 + i
                                hs = slice(512 * i, 512 * i + 512)
                                nc.tensor.matmul(
                                    ps_y[:], v_sb[:, b, :], pT[:, hs],
                                    start=(b == 0), stop=(b == nblk - 1),
                                )

                        for gr in range(ngr):
                            b0 = 2 * gr
                            ps_s = pss.tile([128, 1024], F32, tag="pss")
                            for i in range(2):
                                nc.tensor.matmul(
                                    ps_s[:, 512 * i : 512 * i + 512],
                                    khT[:, 128 * (b0 + i) : 128 * (b0 + i) + 128],
                                    qhT[:, hq, tqs],
                                    start=True, stop=True,
                                )
                            pT = pt_pool.tile([128, 1024], BF16, tag="pt")
                            nc.scalar.activation(
                                out=pT[:], in_=ps_s[:], func=AF.Exp
                            )
                            for i in range(2):
                                d = b0 + i - 4 * tqc
                                if d >= 0:
                                    hs = slice(512 * i, 512 * i + 512)
                                    nc.vector.tensor_mul(
                                        pT[:, hs], pT[:, hs], masks_sb[:, d, :]
                                    )
                            pending.append((pT, b0))
                            if len(pending) > 1:
                                flush()
                        while pending:
                            flush()

                        rr32 = rr2.tile([1, 512], F32, tag="rr32")
                        nc.vector.reciprocal_approx_fast(out=rr32[:], in_=ps_rs[:])
                        rrbf = rr2.tile([1, 512], BF16, tag="rrbf")
                        nc.vector.tensor_copy(out=rrbf[:], in_=rr32[:])
                        y_sb = o_pool.tile([128, 512], BF16, tag="ysb", bufs=8)
                        nc.vector.tensor_copy(out=y_sb[:], in_=ps_y[:])
                        bc = bc2.tile([128, 512], BF16, tag="bc")
                        nc.gpsimd.partition_broadcast(bc[:], rrbf[:])
                        yT = o_pool.tile([128, 512], BF16, tag="ybf", bufs=8)
                        nc.vector.tensor_mul(yT[:], y_sb[:], bc[:])
                        if hq in (1, 2):
                            nc.gpsimd.dma_start(
                                out=ag_in[128 * hq : 128 * hq + 128, tqs],
                                in_=yT[:],
                            )
                        else:
                            ins_t = ag_in0 if hq == 0 else ag_in3
                            outs_t = ag_out0 if hq == 0 else ag_out3
                            nc.gpsimd.dma_start(
                                out=ins_t[tqc // 2][
                                    :, 512 * (tqc % 2) : 512 * (tqc % 2) + 512
                                ],
                                in_=yT[:],
                            )
                            if tqc % 2 == 1:
                                nc.gpsimd.collective_compute(
                                    "AllGather",
                                    mybir.AluOpType.bypass,
                                    replica_groups=[[0, 1, 2, 3], [4, 5, 6, 7]],
                                    ins=[ins_t[tqc // 2][:]],
                                    outs=[outs_t[tqc // 2][:]],
                                )
                    if hq in (1, 2):
                        nc.gpsimd.collective_compute(
                            "AllGather",
                            mybir.AluOpType.bypass,
                            replica_groups=[[0, 1, 2, 3], [4, 5, 6, 7]],
                            ins=[ag_in[HD * hq : HD * hq + HD, :]],
                            outs=[ag_outs[hq][:]],
                        )

                accs = {}

                def proj_pass(qs, last):
                    for tch5 in range(4):
                        tqs = slice(512 * tch5, 512 * tch5 + 512)
                        yts = {}
                        for q in qs:
                            for r in range(4):
                                yt = yt_pool.tile(
                                    [128, 512], BF16, tag="yt",
                                    name=f"yt{q}_{tch5}_{r}",
                                )
                                if q in (1, 2):
                                    src = ag_outs[q][
                                        128 * r : 128 * r + 128, tqs
                                    ]
                                else:
                                    half_t = ag_out0 if q == 0 else ag_out3
                                    hk = tch5 // 2
                                    hs2 = slice(
                                        512 * (tch5 % 2), 512 * (tch5 % 2) + 512
                                    )
                                    src = half_t[hk][
                                        128 * r : 128 * r + 128, hs2
                                    ]
                                nc.sync.dma_start(out=yt[:], in_=src)
                                yts[(q, r)] = yt
                        for cob in range(4):
                            ps_o = pss.tile([128, 512], F32, tag="pss", name="ps_o")
                            n = len(qs) * 4
                            j = 0
                            for q in qs:
                                for r in range(4):
                                    nc.tensor.matmul(
                                        ps_o[:],
                                        wp_sb[
                                            :, 4 * r + q,
                                            128 * cob : 128 * cob + 128,
                                        ],
                                        yts[(q, r)][:],
                                        start=(j == 0), stop=(j == n - 1),
                                    )
                                    j += 1
                            if not last:
                                acc = acc_pool.tile(
                                    [128, 512], BF16, tag="acc",
                                    name=f"acc{tch5}_{cob}",
                                )
                                nc.vector.tensor_copy(out=acc[:], in_=ps_o[:])
                                accs[(tch5, cob)] = acc
                            else:
                                o32 = o_pool.tile([128, 512], F32, tag="o32")
                                nc.vector.tensor_add(
                                    o32[:], accs[(tch5, cob)][:], ps_o[:]
                                )
                                nc.scalar.dma_start(
                                    out=outT[128 * cob : 128 * cob + 128, tqs],
                                    in_=o32[:],
                                )

                attn_head(0)
                attn_head(1)
                attn_head(2)
                proj_pass((0, 1), last=False)
                attn_head(3)
                proj_pass((2, 3), last=True)

    nc.compile()
    return nc


def _get_nc():
    if "nc" not in _CACHE:
        _CACHE["nc"] = _build()
    return _CACHE["nc"]


def build_in_maps(x, cos, sin, Wq, Wk, Wv, Wp):
    x = np.asarray(x, dtype=np.float32)
    cos = np.asarray(cos, dtype=np.float32)
    sin = np.asarray(sin, dtype=np.float32)
    cosb = np.vstack([cos.T, cos.T]).astype(BF)  # [128, T]
    sinb = np.vstack([-sin.T, sin.T]).astype(BF)
    p = np.arange(128, dtype=np.int64)[:, None]
    j = np.arange(512, dtype=np.int64)[None, :]
    masks = np.stack([(j >= p + 128 * d) for d in range(4)], axis=0).astype(BF)

    def dr_pack(a):  # [C, M] -> [NKB, 128, 2, M]
        Cdim, M = a.shape
        return np.ascontiguousarray(
            a.reshape(NKB, 2, 128, M).transpose(0, 2, 1, 3)
        )

    in_maps = []
    for core in range(8):
        b, g = core // 4, core % 4
        xb = np.ascontiguousarray(x[b].T)  # [C, T]
        x8 = dr_pack(xb).astype(F8)
        wq8 = dr_pack(np.asarray(Wq)[:, 512 * g : 512 * g + 512] * WS).astype(F8)
        wk8 = dr_pack(np.asarray(Wk)[:, 128 * g : 128 * g + 128] * WS).astype(F8)
        in_maps.append(
            {
                "x8": x8,
                "xT": xb.astype(BF),
                "wq8": wq8,
                "wk8": wk8,
                "wv": np.ascontiguousarray(
                    np.asarray(Wv)[:, 128 * g : 128 * g + 128]
                ).astype(BF),
                "wp": np.ascontiguousarray(
                    np.asarray(Wp)[:, 512 * g : 512 * g + 512]
                ).astype(BF),
                "cosb": cosb,
                "sinb": sinb,
                "masks": masks,
            }
        )
    return in_maps


def kernel(x, cos, sin, Wq, Wk, Wv, Wp):
    from concourse.bass_utils import run_bass_kernel_spmd

    in_maps = build_in_maps(x, cos, sin, Wq, Wk, Wv, Wp)
    nc = _get_nc()
    res = run_bass_kernel_spmd(nc, in_maps, core_ids=list(range(8)), trace=False)

    out = np.empty((B, T, C), dtype=np.float32)
    for core in range(8):
        b, g = core // 4, core % 4
        out[b, :, 512 * g : 512 * g + 512] = res.results[core]["outT"].T
    return out
